# revision 8
# baseline (speedup 1.0000x reference)
"""DiagonalLSTMCell Trainium2 kernel.

Full inputs -> full output. Internally: batch-parallel over 8 NeuronCores
(B=16 -> 2 images/core). Per core, the 2W-1=127-step diagonal scan runs as a
serial chain; per step:
  - 12 matmuls accumulate z = W_is@x_diag + W_ss(*)h into one PSUM bank
    [128 part = gate-channel(Hd), 512 free = 4 gate slabs x (j,b)]
    (x-part fp32, recurrent taps fp16 - fp32 matmul is 4 cyc/row on TRN2)
  - 3 sigmoid ACT ops (priority-split), with the g-gate pre-scaled by -2 in
    the weights so tanh(zg) = 1 - 2*sigmoid(-2 zg) needs no separate tanh
  - 4 DVE ops implement the cell with state ct = -c/2:
      p = (sg' - 0.5) * si          [= -i*g/2]
      q = sf * ct                   [= -f*c/2]
      ct' = p + q                   [= -c'/2]
      u = tanh(-2*ct')              [ACT, scale immediate]
      h = so * u  -> written into the zero-padded output slab ring, which
                     doubles as the matmul rhs for the next step (the k=0
                     causal tap along H is a 2-column AP offset).
Host does the skew/unskew and layout packing (pure data movement).
"""

import os
import sys

sys.path.insert(0, "/opt/trn_rl_repo")
os.environ.setdefault("MYCRO_LOCAL_CACHE", "1")

import numpy as np

# problem constants (hardcoded per contract)
B, CIN, H, W = 16, 64, 64, 64
HD = 128
T = 2 * W - 1  # 127
NCORES = 8
BL = B // NCORES  # 2 images per core
JB = H * BL  # 128 = (j,b) columns per step
SLAB = JB + 2  # 130: 2 zero pad cols (j=-1 boundary) + 128 h cols
NBLK = T + 1  # 128 slabs: s = -1..126

_CACHE = {}


def build_program(t_steps=T, dma_chunk=16, fp16_x=False, no_xmm=False, probe_no_tanh=False, probe_one_dve=False):
    import concourse.bass as bass
    import concourse.tile as tile
    from concourse import bacc, mybir

    f32 = mybir.dt.float32
    f16 = mybir.dt.float16
    AF = mybir.ActivationFunctionType
    OP = mybir.AluOpType

    nc = bacc.Bacc(
        "TRN2",
        target_bir_lowering=False,
        debug=False,
        enable_asserts=False,
        num_devices=NCORES,
    )

    fx = f16 if fp16_x else f32
    nblk = t_steps + 1
    xs_d = nc.dram_tensor("xs", [CIN + 1, H * BL * T], fx, kind="ExternalInput").ap()
    wt_d = nc.dram_tensor("wt", [HD, 8 * JB], f16, kind="ExternalInput").ap()
    wx_d = nc.dram_tensor("wx", [CIN + 1, 4 * JB], fx, kind="ExternalInput").ap()
    h0_d = nc.dram_tensor("h0", [HD, JB], f16, kind="ExternalInput").ap()
    c0_d = nc.dram_tensor("c0s", [HD, JB], f32, kind="ExternalInput").ap()
    out_d = nc.dram_tensor("out", [HD, t_steps * JB], f16, kind="ExternalOutput").ap()

    with tile.TileContext(nc) as tc:
        with (
            tc.tile_pool(name="persist", bufs=1) as pp,
            tc.tile_pool(name="psum", bufs=2, space=bass.MemorySpace.PSUM) as psp,
        ):
            xs = pp.tile([CIN + 1, H * BL * T], fx, tag="xs")
            wt = pp.tile([HD, 8 * JB], f16, tag="wt")
            wx = pp.tile([CIN + 1, 4 * JB], fx, tag="wx")
            outb = pp.tile([HD, NBLK * SLAB], f16, tag="outb")
            ct = pp.tile([HD, JB], f32, tag="ct")
            S = pp.tile([HD, 4 * JB], f32, tag="S")
            p_t = pp.tile([HD, JB], f32, tag="p")
            q_t = pp.tile([HD, JB], f32, tag="q")
            u_t = pp.tile([HD, JB], f32, tag="u")

            nc.sync.dma_start(xs[:], xs_d)
            nc.sync.dma_start(wt[:], wt_d)
            nc.sync.dma_start(wx[:], wx_d)
            # zero the 2-col pads of every slab, then land h0 in slab -1
            outb_r = outb[:].rearrange("p (s c) -> p s c", s=NBLK, c=SLAB)
            nc.vector.memset(outb_r[:, :, 0:2], 0.0)
            nc.sync.dma_start(outb[:, 2:SLAB], h0_d)
            nc.sync.dma_start(ct[:], c0_d)

            xs_r = xs[:].rearrange("p (j b t) -> p j b t", j=H, b=BL, t=T)
            out_r = out_d.rearrange("p (s c) -> p s c", s=t_steps, c=JB)

            # gate slab order: [i, g', f, o]
            for t in range(t_steps):
                bp = t * SLAB  # slab t-1 base
                bc = (t + 1) * SLAB  # slab t base
                zt = [
                    psp.tile([HD, JB], f32, tag=f"z{g}", name=f"z{g}")
                    for g in range(4)
                ]
                xr = xs_r[:, :, :, t]  # [65, j, b] strided diag slice
                # x-contribution (independent of h -> fills PE idle time)
                if not no_xmm:
                    for g in range(4):
                        nc.tensor.matmul(
                            zt[g][:],
                            wx[:, g * JB : (g + 1) * JB],
                            xr,
                            start=True,
                            stop=False,
                        )
                rhs1 = outb[:, bp + 2 : bp + SLAB]  # h(t-1)[j]
                rhs0 = outb[:, bp : bp + JB]  # h(t-1)[j-1] (2-col shift)
                for g in range(4):
                    nc.tensor.matmul(
                        zt[g][:],
                        wt[:, (2 * g) * JB : (2 * g + 1) * JB],
                        rhs1,
                        start=no_xmm,
                        stop=False,
                    )
                    nc.tensor.matmul(
                        zt[g][:],
                        wt[:, (2 * g + 1) * JB : (2 * g + 2) * JB],
                        rhs0,
                        start=False,
                        stop=True,
                    )
                    nc.scalar.activation(
                        S[:, g * JB : (g + 1) * JB], zt[g][:], AF.Sigmoid
                    )
                # p = (sigma'g - 0.5) * sigma_i  = -i*g/2
                nc.vector.scalar_tensor_tensor(
                    ct[:] if probe_one_dve else p_t[:],
                    S[:, JB : 2 * JB], 0.5, S[:, 0:JB], OP.subtract, OP.mult
                )
                if not probe_one_dve:
                    # q = sigma_f * ct = -f*c/2
                    nc.vector.tensor_tensor(q_t[:], S[:, 2 * JB : 3 * JB], ct[:], OP.mult)
                    # ct' = p + q = -c'/2
                    nc.vector.tensor_tensor(ct[:], p_t[:], q_t[:], OP.add)
                # u = tanh(-2*ct') = tanh(c')
                if not probe_no_tanh:
                    nc.scalar.activation(u_t[:], ct[:], AF.Tanh, scale=-2.0)
                # h = sigma_o * u -> slab t (fp16)
                nc.vector.tensor_tensor(
                    outb[:, bc + 2 : bc + SLAB], S[:, 3 * JB : 4 * JB],
                    ct[:] if probe_no_tanh else u_t[:], OP.mult
                )
                if (t + 1) % dma_chunk == 0 or t == t_steps - 1:
                    s1 = t + 1
                    s0 = (t // dma_chunk) * dma_chunk
                    nc.sync.dma_start(
                        out_r[:, s0:s1, :], outb_r[:, s0 + 1 : s1 + 1, 2:SLAB]
                    )

    nc.compile()
    return nc


def _host_pack(x, h0, c0, W_is, b_is, W_ss, b_ss):
    """Pack full inputs into per-core in_maps."""
    assert np.allclose(np.asarray(b_ss), 0.0), "kernel assumes b_ss == 0"
    x = np.asarray(x, np.float32)
    h0 = np.asarray(h0, np.float32)
    c0 = np.asarray(c0, np.float32)
    W_is = np.asarray(W_is, np.float32)
    W_ss = np.asarray(W_ss, np.float32)
    b_is = np.asarray(b_is, np.float32)

    # gate permutation [i, g, f, o] over the 4*HD=512 output channels
    perm = np.r_[0:HD, 3 * HD : 4 * HD, HD : 2 * HD, 2 * HD : 3 * HD]
    Wss_p = W_ss[perm].copy()
    Wis_p = W_is[perm].copy()
    b_p = b_is[perm].copy()
    Wss_p[HD : 2 * HD] *= -2.0  # g-gate folded: sigma(-2 zg)
    Wis_p[HD : 2 * HD] *= -2.0
    b_p[HD : 2 * HD] *= -2.0

    wt = np.zeros((HD, 8 * JB), np.float16)
    for g in range(4):
        for ki, k in enumerate((1, 0)):
            wt[:, (2 * g + ki) * JB : (2 * g + ki + 1) * JB] = Wss_p[
                g * HD : (g + 1) * HD, :, k
            ].T.astype(np.float16)
    wx = np.zeros((CIN + 1, 4 * JB), np.float32)
    for g in range(4):
        wx[0:CIN, g * JB : (g + 1) * JB] = Wis_p[g * HD : (g + 1) * HD, :].T
        wx[CIN, g * JB : (g + 1) * JB] = b_p[g * HD : (g + 1) * HD]

    # xs[core, c, j, b, t] = x[2*core+b, c, j, t-j]; ones row (valid-masked) for bias
    xv = x.reshape(NCORES, BL, CIN, H, W)
    xs = np.zeros((NCORES, CIN + 1, H, BL, T), np.float32)
    for j in range(H):
        xs[:, 0:CIN, j, :, j : j + W] = xv[:, :, :, j, :].transpose(0, 2, 1, 3)
        xs[:, CIN, j, :, j : j + W] = 1.0
    xs = xs.reshape(NCORES, CIN + 1, H * BL * T)

    # h0/c0: [B, Hd, H, 1] -> per-core [Hd, (j,b)]
    h0v = h0.reshape(NCORES, BL, HD, H).transpose(0, 2, 3, 1).reshape(NCORES, HD, JB)
    c0v = c0.reshape(NCORES, BL, HD, H).transpose(0, 2, 3, 1).reshape(NCORES, HD, JB)
    in_maps = []
    for m in range(NCORES):
        in_maps.append(
            {
                "xs": np.ascontiguousarray(xs[m]),
                "wt": wt,
                "wx": wx,
                "h0": np.ascontiguousarray(h0v[m]).astype(np.float16),
                "c0s": np.ascontiguousarray(-0.5 * c0v[m]),
            }
        )
    return in_maps


def _host_unpack(outs):
    """outs: per-core [HD, T*JB] fp16 -> full [B, HD, H, W] fp32 (unskew)."""
    full = np.zeros((B, HD, H, W), np.float32)
    j = np.arange(H)[:, None]
    w = np.arange(W)[None, :]
    t = j + w  # [H, W]
    for m in range(NCORES):
        o = np.asarray(outs[m], np.float32).reshape(HD, T, JB)
        # full[2m+b, hd, j, w] = o[hd, j+w, j*BL+b]
        for b in range(BL):
            full[BL * m + b] = o[:, t, j * BL + b]
    return full


def kernel(x, h0, c0, W_is, b_is, W_ss, b_ss):
    from concourse import bass_utils

    if "nc" not in _CACHE:
        _CACHE["nc"] = build_program()
    nc = _CACHE["nc"]
    in_maps = _host_pack(x, h0, c0, W_is, b_is, W_ss, b_ss)
    res = bass_utils.run_bass_kernel_spmd(
        nc,
        in_maps,
        core_ids=list(range(NCORES)),
        trace=bool(int(os.environ.get("KERNEL_TRACE", "0"))),
    )
    _CACHE["last_results"] = res
    return _host_unpack([r["out"] for r in res.results])


# revision 9
# speedup vs baseline: 3338.7491x; 3338.7491x over previous
"""DiagonalLSTMCell TRN2 kernel (all-sigmoid split-chain design): split per-batch chains (b=0,1) interleaved to hide chain latency.

Column order is (b, j): col = b*64 + j. Each chain owns a contiguous
64-col half of every tile. OUT slab = [pad|b0 h(64)|pad|b1 h(64)] = 130 cols.
One shared PSUM tile [128, 2048] (4 banks, slab g in bank g); chain A's x-MM
carries start=True (bank-wide has_written clear), chain B's last tap stops.
"""

import os
import sys

sys.path.insert(0, "/opt/trn_rl_repo")
os.environ.setdefault("MYCRO_LOCAL_CACHE", "1")

import numpy as np

B, CIN, H, W = 16, 64, 64, 64
HD = 128
T = 2 * W - 1
NCORES = 8
BL = B // NCORES  # 2 chains per core
JB = H * BL  # 128
CW = H  # 64 cols per chain
SLAB = 2 * (CW + 1)  # 130
NBLK = T + 1

_CACHE = {}
OUT_SCALE = 2.0


def build_program(t_steps=T, dma_chunk=16):
    import concourse.bass as bass
    import concourse.tile as tile
    from concourse import bacc, mybir

    f32 = mybir.dt.float32
    f16 = mybir.dt.float16
    AF = mybir.ActivationFunctionType
    OP = mybir.AluOpType

    nc = bacc.Bacc(
        "TRN2",
        target_bir_lowering=False,
        debug=False,
        enable_asserts=False,
        num_devices=NCORES,
    )

    xs_d = nc.dram_tensor("xs", [CIN + 1, BL * H * T], f32, kind="ExternalInput").ap()
    wt_d = nc.dram_tensor("wt", [HD, 8 * HD], f16, kind="ExternalInput").ap()
    wx_d = nc.dram_tensor("wx", [CIN + 1, 4 * HD], f32, kind="ExternalInput").ap()
    h0_d = nc.dram_tensor("h0", [HD, JB], f16, kind="ExternalInput").ap()
    c0_d = nc.dram_tensor("c0s", [HD, JB], f32, kind="ExternalInput").ap()
    out_d = nc.dram_tensor("out", [HD, t_steps * JB], f16, kind="ExternalOutput").ap()

    with tile.TileContext(nc) as tc:
        with (
            tc.tile_pool(name="persist", bufs=1) as pp,
            tc.tile_pool(name="psum", bufs=2, space=bass.MemorySpace.PSUM) as psp,
        ):
            xs = pp.tile([CIN + 1, BL * H * T], f32, tag="xs")
            wt = pp.tile([HD, 8 * HD], f16, tag="wt")
            wx = pp.tile([CIN + 1, 4 * HD], f32, tag="wx")
            outb = pp.tile([HD, NBLK * SLAB], f16, tag="outb")
            ct = pp.tile([HD, JB], f32, tag="ct")
            S = pp.tile([HD, 4 * JB], f32, tag="S")
            p_t = pp.tile([HD, JB], f32, tag="p")
            q_t = pp.tile([HD, JB], f32, tag="q")
            u_t = pp.tile([HD, JB], f32, tag="u")

            # chunk the big x DMA by t-range so early steps start sooner
            xs_rt = xs[:].rearrange("p (b j t) -> p b j t", b=BL, j=H, t=T)
            xd_rt = xs_d.rearrange("p (b j t) -> p b j t", b=BL, j=H, t=T)
            for k in range(8):
                t0c, t1c = k * 16, min((k + 1) * 16, T)
                nc.sync.dma_start(
                    xs_rt[:, :, :, t0c:t1c], xd_rt[:, :, :, t0c:t1c]
                )
            nc.sync.dma_start(wt[:], wt_d)
            nc.sync.dma_start(wx[:], wx_d)
            outb_r = outb[:].rearrange("p (s c) -> p s c", s=NBLK, c=SLAB)
            outb_rb = outb[:].rearrange(
                "p (s b c) -> p s b c", s=NBLK, b=BL, c=CW + 1
            )
            nc.vector.memset(outb_rb[:, :, :, 0:1], 0.0)
            # h0 -> slab -1: dst cols [1:65] and [66:130]
            nc.sync.dma_start(outb_rb[:, 0, :, 1 : CW + 1], h0_d)
            nc.sync.dma_start(ct[:], c0_d)

            out_r = out_d.rearrange("p (s c) -> p s c", s=t_steps, c=JB)
            z_r = lambda z: z[:].rearrange("p (g x) -> p g x", g=4, x=512)
            S_r = S[:].rearrange("p (g x) -> p g x", g=4, x=JB)

            for t in range(t_steps):
                bp = t * SLAB
                bc = (t + 1) * SLAB
                z = psp.tile([HD, 4 * 512], f32, tag="z", name="z")
                zr = z_r(z)
                for c in range(BL):
                    xr = xs[:, c * H * T + t : (c + 1) * H * T : T]  # [65, 64] diag
                    for g in range(4):
                        nc.tensor.matmul(
                            zr[:, g, c * CW : (c + 1) * CW],
                            wx[:, g * HD : (g + 1) * HD],
                            xr,
                            start=(c == 0),
                            stop=False,
                            skip_group_check=True,
                        )
                    co = bp + c * (CW + 1)
                    rhs1 = outb[:, co + 1 : co + 1 + CW]
                    rhs0 = outb[:, co : co + CW]
                    for g in range(4):
                        nc.tensor.matmul(
                            zr[:, g, c * CW : (c + 1) * CW],
                            wt[:, (2 * g) * HD : (2 * g + 1) * HD],
                            rhs1,
                            start=False,
                            stop=False,
                            skip_group_check=True,
                        )
                        nc.tensor.matmul(
                            zr[:, g, c * CW : (c + 1) * CW],
                            wt[:, (2 * g + 1) * HD : (2 * g + 2) * HD],
                            rhs0,
                            start=False,
                            stop=(c == BL - 1),
                            skip_group_check=True,
                        )
                    # one sigmoid over all 4 gate slabs of this chain
                    nc.scalar.activation(
                        S_r[:, :, c * CW : (c + 1) * CW],
                        zr[:, :, c * CW : (c + 1) * CW],
                        AF.Sigmoid,
                    )
                for c in range(BL):
                    cs = slice(c * CW, (c + 1) * CW)
                    nc.vector.scalar_tensor_tensor(
                        p_t[:, cs],
                        S[:, JB + c * CW : JB + (c + 1) * CW],
                        0.5,
                        S[:, c * CW : (c + 1) * CW],
                        OP.subtract,
                        OP.mult,
                    )
                    nc.vector.tensor_tensor(
                        q_t[:, cs], S[:, 2 * JB + c * CW : 2 * JB + (c + 1) * CW],
                        ct[:, cs], OP.mult,
                    )
                    nc.vector.tensor_tensor(ct[:, cs], p_t[:, cs], q_t[:, cs], OP.add)
                for c in range(BL):
                    cs = slice(c * CW, (c + 1) * CW)
                    nc.scalar.activation(u_t[:, cs], ct[:, cs], AF.Tanh, scale=-2.0)
                for c in range(BL):
                    cs = slice(c * CW, (c + 1) * CW)
                    co = bc + c * (CW + 1)
                    nc.vector.tensor_tensor(
                        outb[:, co + 1 : co + 1 + CW],
                        S[:, 3 * JB + c * CW : 3 * JB + (c + 1) * CW],
                        u_t[:, cs],
                        OP.mult,
                    )
                if (t + 1) % dma_chunk == 0 or t == t_steps - 1:
                    s1 = t + 1
                    s0 = (t // dma_chunk) * dma_chunk
                    nc.sync.dma_start(
                        out_r[:, s0:s1, :],
                        outb_rb[:, s0 + 1 : s1 + 1, :, 1 : CW + 1],
                    )

    nc.compile()
    return nc


def _host_pack(x, h0, c0, W_is, b_is, W_ss, b_ss):
    assert np.allclose(np.asarray(b_ss), 0.0), "kernel assumes b_ss == 0"
    x = np.asarray(x, np.float32)
    h0 = np.asarray(h0, np.float32)
    c0 = np.asarray(c0, np.float32)
    W_is = np.asarray(W_is, np.float32)
    W_ss = np.asarray(W_ss, np.float32)
    b_is = np.asarray(b_is, np.float32)

    perm = np.r_[0:HD, 3 * HD : 4 * HD, HD : 2 * HD, 2 * HD : 3 * HD]
    Wss_p = W_ss[perm].copy()
    Wis_p = W_is[perm].copy()
    b_p = b_is[perm].copy()
    Wss_p[HD : 2 * HD] *= -2.0
    Wis_p[HD : 2 * HD] *= -2.0
    b_p[HD : 2 * HD] *= -2.0

    wt = np.zeros((HD, 8 * HD), np.float16)
    for g in range(4):
        for ki, k in enumerate((1, 0)):
            wt[:, (2 * g + ki) * HD : (2 * g + ki + 1) * HD] = (
                2.0 * Wss_p[g * HD : (g + 1) * HD, :, k]
            ).T.astype(np.float16)
    wx = np.zeros((CIN + 1, 4 * HD), np.float32)
    for g in range(4):
        wx[0:CIN, g * HD : (g + 1) * HD] = Wis_p[g * HD : (g + 1) * HD, :].T
        wx[CIN, g * HD : (g + 1) * HD] = b_p[g * HD : (g + 1) * HD]

    # xs[core, c, b, j, t] = x[2*core+b, c, j, t-j]
    xv = x.reshape(NCORES, BL, CIN, H, W)
    xs = np.zeros((NCORES, CIN + 1, BL, H, T), np.float32)
    for j in range(H):
        xs[:, 0:CIN, :, j, j : j + W] = xv[:, :, :, j, :].transpose(0, 2, 1, 3)
        xs[:, CIN, :, j, j : j + W] = 1.0
    xs = xs.reshape(NCORES, CIN + 1, BL * H * T)

    # h0/c0: [B, Hd, H, 1] -> [core, Hd, b*64+j]
    h0v = h0.reshape(NCORES, BL, HD, H).transpose(0, 2, 1, 3).reshape(NCORES, HD, JB)
    c0v = c0.reshape(NCORES, BL, HD, H).transpose(0, 2, 1, 3).reshape(NCORES, HD, JB)
    in_maps = []
    for m in range(NCORES):
        in_maps.append(
            {
                "xs": np.ascontiguousarray(xs[m]),
                "wt": wt,
                "wx": wx,
                "h0": np.ascontiguousarray(0.5 * h0v[m]).astype(np.float16),
                "c0s": np.ascontiguousarray(-0.5 * c0v[m]),
            }
        )
    return in_maps


def _host_unpack(outs):
    full = np.zeros((B, HD, H, W), np.float32)
    j = np.arange(H)[:, None]
    w = np.arange(W)[None, :]
    t = j + w
    for m in range(NCORES):
        o = np.asarray(outs[m], np.float32).reshape(HD, T, JB)
        for b in range(BL):
            full[BL * m + b] = 2.0 * o[:, t, b * CW + j]
    return full


def kernel(x, h0, c0, W_is, b_is, W_ss, b_ss):
    from concourse import bass_utils

    if "nc" not in _CACHE:
        _CACHE["nc"] = build_program()
    nc = _CACHE["nc"]
    in_maps = _host_pack(x, h0, c0, W_is, b_is, W_ss, b_ss)
    res = bass_utils.run_bass_kernel_spmd(
        nc, in_maps, core_ids=list(range(NCORES))
    )
    _CACHE["last_results"] = res
    return _host_unpack([r["out"] for r in res.results])
